# revision 4
# baseline (speedup 1.0000x reference)
"""Trainium2 Bass kernel for: out = 0.5 * sum_g maxpool4(x @ W.T + b).

Shapes: x [4096, 2048] f32, W [4096, 2048] f32, b [4096] f32 -> out [4096] f32.

Sharding over 8 NeuronCores: 2 batch-groups x 4 out-feature-groups.
Core c = (g, j): batch rows g*2048:(g+1)*2048, out features j*1024:(j+1)*1024.
Each core computes partial row-sums of its pooled quarter; host adds the 4
out-feature partials per batch half (pooling groups of 4 are never split
across cores since 1024 % 4 == 0).

Per-core kernel: y tile layout [batch=128 partitions, out_f=512 free].
  lhsT (stationary) = x^T k-slice [128 i, 128 b], rhs (moving) = W^T k-slice
  [128 i, 512 o], accumulating over 16 k-slices into PSUM fp32. Bias is added
  via one extra K=1 matmul (ones[1,128] x bias[1,512]) per accumulation
  group. MaxPool4 + row-sum on VectorE via 3D-AP tensor_reduce. The 0.5
  output scale is folded into W and b on the host (max is monotone under
  positive scaling). Inputs are cast to bf16 on host (PE runs bf16 at 1
  cycle/row vs 4 for fp32); PSUM accumulation stays fp32.

Loop order: k-major over groups of 4 batch-tiles (8 PSUM banks = 4 b x 2 o)
so each W^T k-slice DMA is consumed by 8 back-to-back matmuls and the PE
never waits on HBM after the first slice. Everything (12 MiB/core) stays
resident in SBUF.
"""

import sys

if "/opt/trn_rl_repo" not in sys.path:
    sys.path.insert(0, "/opt/trn_rl_repo")

import numpy as np
import ml_dtypes

# Problem constants (hardcoded per harness contract).
B, I, O = 4096, 2048, 4096
KS = 4  # maxpool kernel size
SCALE = 0.5
NB_G, NO_G = 2, 4  # batch groups x out-feature groups = 8 cores
BC = B // NB_G  # 2048 batch rows per core
OC = O // NO_G  # 1024 out features per core
P = 128
KT = I // P  # 16 contraction slices
BT = BC // P  # 16 batch tiles per core
GRP = 4  # batch tiles per k-major group
NG = BT // GRP  # 4 groups
NFREE = 512  # matmul moving free dim (one PSUM bank fp32)
OT = OC // NFREE  # 2 out-feature tiles of 512 per core

_NC_CACHE = {}


def build_bass():
    """Build the (SPMD, per-core) Bass program."""
    from concourse import bacc, tile
    import concourse.mybir as mybir

    f32 = mybir.dt.float32
    bf16 = mybir.dt.bfloat16

    # Bacc (not plain Bass): its compile() runs the TRN2 legalization passes
    # (move_matmul_waits_to_ldweights, generate_event_semaphores) without
    # which walrus rejects matmuls carrying >1 semaphore wait.
    nc = bacc.Bacc(
        "TRN2", target_bir_lowering=False, debug=False, num_devices=NB_G * NO_G
    )
    xt_d = nc.dram_tensor("xt", [KT, NG, P, GRP * P], bf16, kind="ExternalInput")
    wt_d = nc.dram_tensor("wt", [KT, P, OC], bf16, kind="ExternalInput")
    bias_d = nc.dram_tensor("bias", [1, OC], bf16, kind="ExternalInput")
    out_d = nc.dram_tensor("out", [P, BT], f32, kind="ExternalOutput")

    with tile.TileContext(nc) as tc:
        with (
            tc.tile_pool(name="wt", bufs=KT) as wt_pool,
            tc.tile_pool(name="xt", bufs=KT * NG) as xt_pool,
            tc.tile_pool(name="misc", bufs=1) as misc_pool,
            tc.tile_pool(name="pooled", bufs=4) as pooled_pool,
            tc.tile_pool(name="psum", bufs=8, space="PSUM") as psum_pool,
        ):
            ones = misc_pool.tile([1, P], bf16)
            nc.vector.memset(ones[:], 1.0)
            bias_sb = misc_pool.tile([1, OC], bf16)
            nc.sync.dma_start(bias_sb[:], bias_d[0, :])
            outsb = misc_pool.tile([P, BT], f32)

            # Loads, in PE consumption order: group 0 needs wt[k] + xt[k,0]
            # pairs; later groups' xt slabs stream behind.
            wt_sb = []
            xt_sb = {}
            for k in range(KT):
                w = wt_pool.tile([P, OC], bf16, tag="wt")
                nc.sync.dma_start(w[:], wt_d[k, :, :])
                wt_sb.append(w)
                xx = xt_pool.tile([P, GRP * P], bf16, tag="xt")
                nc.sync.dma_start(xx[:], xt_d[k, 0, :, :])
                xt_sb[(k, 0)] = xx
            for g in range(1, NG):
                for k in range(KT):
                    xx = xt_pool.tile([P, GRP * P], bf16, tag="xt")
                    nc.sync.dma_start(xx[:], xt_d[k, g, :, :])
                    xt_sb[(k, g)] = xx

            for g in range(NG):
                psums = [
                    [
                        psum_pool.tile([P, NFREE], f32, tag="ps", name=f"ps_{g}_{bb}_{o}")
                        for o in range(OT)
                    ]
                    for bb in range(GRP)
                ]
                for k in range(KT):
                    for bb in range(GRP):
                        lhsT = xt_sb[(k, g)][:, bb * P : (bb + 1) * P]
                        for o in range(OT):
                            nc.tensor.matmul(
                                psums[bb][o][:],
                                lhsT,
                                wt_sb[k][:, o * NFREE : (o + 1) * NFREE],
                                start=(k == 0),
                                stop=False,
                            )
                for bb in range(GRP):
                    for o in range(OT):
                        nc.tensor.matmul(
                            psums[bb][o][:],
                            ones[:],
                            bias_sb[:, o * NFREE : (o + 1) * NFREE],
                            start=False,
                            stop=True,
                        )
                for bb in range(GRP):
                    pooled = pooled_pool.tile([P, OT, P], f32, tag="pooled")
                    for o in range(OT):
                        nc.vector.reduce_max(
                            pooled[:, o, :],
                            psums[bb][o][:].rearrange("p (q f) -> p q f", f=KS),
                            axis=mybir.AxisListType.X,
                        )
                    col = g * GRP + bb
                    nc.vector.reduce_sum(
                        outsb[:, col : col + 1],
                        pooled[:, :, :],
                        axis=mybir.AxisListType.XY,
                    )

            nc.sync.dma_start(out_d[:, :], outsb[:])

    nc.compile()
    return nc


def make_in_maps(x, W, b):
    """Host-side shard + preprocess: transpose, fold 0.5, cast bf16."""
    x = np.asarray(x, dtype=np.float32)
    W = np.asarray(W, dtype=np.float32)
    b = np.asarray(b, dtype=np.float32)

    xt = np.ascontiguousarray(x.T).astype(ml_dtypes.bfloat16)  # [I, B]
    wt = np.ascontiguousarray(W.T * np.float32(SCALE)).astype(
        ml_dtypes.bfloat16
    )  # [I, O]
    bias = (b * np.float32(SCALE)).astype(ml_dtypes.bfloat16).reshape(1, O)

    # Per-batch-half x slabs: [KT, NG, P, GRP*P]
    x_slabs = []
    for g in range(NB_G):
        xg = xt[:, g * BC : (g + 1) * BC]  # [I, BC]
        xr = np.ascontiguousarray(
            xg.reshape(KT, P, NG, GRP * P).transpose(0, 2, 1, 3)
        )
        x_slabs.append(xr)
    # Per-out-feature-quarter W slabs: [KT, P, OC]
    w_slabs = []
    b_slabs = []
    for j in range(NO_G):
        w_slabs.append(
            np.ascontiguousarray(wt[:, j * OC : (j + 1) * OC]).reshape(KT, P, OC)
        )
        b_slabs.append(np.ascontiguousarray(bias[:, j * OC : (j + 1) * OC]))

    in_maps = []
    for c in range(NB_G * NO_G):
        g, j = divmod(c, NO_G)
        in_maps.append({"xt": x_slabs[g], "wt": w_slabs[j], "bias": b_slabs[j]})
    return in_maps


def combine_outputs(results):
    """Sum the 4 out-feature partials per batch half -> full [B] output."""
    out = np.zeros(B, dtype=np.float32)
    for c, r in enumerate(results):
        g = c // NO_G
        part = np.asarray(r["out"], dtype=np.float32)  # [P, BT]
        out[g * BC : (g + 1) * BC] += part.T.reshape(BC)
    return out


def kernel(x, W, b):
    from concourse.bass_utils import run_bass_kernel_spmd

    if "nc" not in _NC_CACHE:
        _NC_CACHE["nc"] = build_bass()
    nc = _NC_CACHE["nc"]
    in_maps = make_in_maps(x, W, b)
    res = run_bass_kernel_spmd(nc, in_maps, core_ids=list(range(NB_G * NO_G)))
    return combine_outputs(res.results)


# revision 13
# speedup vs baseline: 1.1960x; 1.1960x over previous
"""Trainium2 Bass kernel for: out = 0.5 * sum_g maxpool4(x @ W.T + b).

Shapes: x [4096, 2048] f32, W [4096, 2048] f32, b [4096] f32 -> out [4096] f32.

Sharding over 8 NeuronCores: 2 batch-groups x 4 out-feature-groups.
Core c = (g, j): batch rows g*2048:(g+1)*2048, out features j*1024:(j+1)*1024.
Each core computes partial row-sums of its pooled quarter; host adds the 4
out-feature partials per batch half (pooling groups of 4 are never split
across cores since 1024 % 4 == 0).

Per-core kernel: y tile layout [batch=128 partitions, out_f=512 free].
  lhsT (stationary) = x^T k-slice [128 i, 128 b], rhs (moving) = W^T k-slice
  [128 i, 512 o], accumulating over 16 k-slices into PSUM fp32. Bias is added
  via one extra K=1 matmul (ones[1,128] x bias[1,512]) per accumulation
  group. MaxPool4 + row-sum on VectorE via 3D-AP tensor_reduce. The 0.5
  output scale is folded into W and b on the host (max is monotone under
  positive scaling). Inputs are cast to bf16 on host (PE runs bf16 at 1
  cycle/row vs 4 for fp32); PSUM accumulation stays fp32.

Loop order: k-major over groups of 4 batch-tiles (8 PSUM banks = 4 b x 2 o)
so each W^T k-slice DMA is consumed by 8 back-to-back matmuls and the PE
never waits on HBM after the first slice. Everything (12 MiB/core) stays
resident in SBUF.
"""

import sys

if "/opt/trn_rl_repo" not in sys.path:
    sys.path.insert(0, "/opt/trn_rl_repo")

import numpy as np
import ml_dtypes

# Problem constants (hardcoded per harness contract).
B, I, O = 4096, 2048, 4096
KS = 4  # maxpool kernel size
SCALE = 0.5
NB_G, NO_G = 2, 4  # batch groups x out-feature groups = 8 cores
BC = B // NB_G  # 2048 batch rows per core
OC = O // NO_G  # 1024 out features per core
P = 128
KT = I // P  # 16 contraction slices
BT = BC // P  # 16 batch tiles per core
GRP = 4  # batch tiles per k-major group
NG = BT // GRP  # 4 groups
NFREE = 512  # matmul moving free dim (one PSUM bank fp32)
OT = OC // NFREE  # 2 out-feature tiles of 512 per core

_NC_CACHE = {}


def _dedup_ldweights(nc):
    """Remove redundant standalone Ldweights from the compiled module.

    bacc splits every Matmult into Ldweights + Matmult(ldweights=False) with
    no dedup, costing ~46 ns/matmul of PE queue time. When consecutive PE
    Ldweights load the identical stationary AP, the array already holds the
    weights, so sync-free duplicates can be dropped. Only duplicates with no
    semaphore waits/updates are removed (a wait-carrying Ldweights guards a
    real dependency).
    """
    removed = 0
    for f in nc.m.functions:
        for blk in f.blocks:
            insts = list(blk.instructions)
            keep = []
            blk_removed = 0
            last_key = None
            for ins in insts:
                tname = type(ins).__name__
                if tname == "InstLdweights":
                    ap = ins.ins[0]
                    key = (
                        ap.memref,
                        ap.offset,
                        str(ap.ap),
                        str(ap.dtype),
                        str(ins.tile_position),
                        str(ins.tile_size),
                        str(ins.perf_mode),
                        str(ins.is_transpose),
                    )
                    if (
                        key == last_key
                        and not ins.has_wait()
                        and not ins.has_update()
                    ):
                        blk_removed += 1
                        continue
                    last_key = key
                keep.append(ins)
            if blk_removed:
                blk.instructions[:] = keep
                removed += blk_removed
    return removed


def build_bass():
    """Build the (SPMD, per-core) Bass program."""
    from concourse import bacc, tile
    import concourse.mybir as mybir

    f32 = mybir.dt.float32
    bf16 = mybir.dt.bfloat16

    # Bacc (not plain Bass): its compile() runs the TRN2 legalization passes
    # (move_matmul_waits_to_ldweights, generate_event_semaphores) without
    # which walrus rejects matmuls carrying >1 semaphore wait.
    nc = bacc.Bacc(
        "TRN2", target_bir_lowering=False, debug=False, num_devices=NB_G * NO_G
    )
    xt_d = nc.dram_tensor("xt", [KT, NG, P, GRP * P], bf16, kind="ExternalInput")
    wt_d = nc.dram_tensor("wt", [KT, P, OC], bf16, kind="ExternalInput")
    bias_d = nc.dram_tensor("bias", [1, OC], bf16, kind="ExternalInput")
    out_d = nc.dram_tensor("out", [P, BT], f32, kind="ExternalOutput")

    with tile.TileContext(nc) as tc:
        with (
            tc.tile_pool(name="wt", bufs=KT) as wt_pool,
            tc.tile_pool(name="xt", bufs=KT * NG) as xt_pool,
            tc.tile_pool(name="misc", bufs=1) as misc_pool,
            tc.tile_pool(name="pooled", bufs=4) as pooled_pool,
            tc.tile_pool(name="psum", bufs=8, space="PSUM") as psum_pool,
        ):
            # Loads, in PE consumption order: group 0 needs wt[k] + xt[k,0]
            # pairs first; bias/ones are only needed at the end of group 0,
            # so they go after the first pair. Later groups' xt slabs stream
            # behind.
            wt_sb = []
            xt_sb = {}
            w = wt_pool.tile([P, OC], bf16, tag="wt", name="w_0")
            nc.sync.dma_start(w[:], wt_d[0, :, :])
            wt_sb.append(w)
            xx = xt_pool.tile([P, GRP * P], bf16, tag="xt", name="xx_0_0")
            nc.sync.dma_start(xx[:], xt_d[0, 0, :, :])
            xt_sb[(0, 0)] = xx

            ones = misc_pool.tile([1, P], bf16)
            nc.vector.memset(ones[:], 1.0)
            bias_sb = misc_pool.tile([1, OC], bf16)
            nc.sync.dma_start(bias_sb[:], bias_d[0, :])
            outsb = misc_pool.tile([P, BT], f32)

            for k in range(1, KT):
                w = wt_pool.tile([P, OC], bf16, tag="wt", name=f"w_{k}")
                nc.sync.dma_start(w[:], wt_d[k, :, :])
                wt_sb.append(w)
                xx = xt_pool.tile([P, GRP * P], bf16, tag="xt", name=f"xx_{k}_0")
                nc.sync.dma_start(xx[:], xt_d[k, 0, :, :])
                xt_sb[(k, 0)] = xx
            for g in range(1, NG):
                for k in range(KT):
                    xx = xt_pool.tile([P, GRP * P], bf16, tag="xt", name=f"xx_{k}_{g}")
                    nc.sync.dma_start(xx[:], xt_d[k, g, :, :])
                    xt_sb[(k, g)] = xx

            for g in range(NG):
                psums = [
                    [
                        psum_pool.tile([P, NFREE], f32, tag="ps", name=f"ps_{g}_{bb}_{o}")
                        for o in range(OT)
                    ]
                    for bb in range(GRP)
                ]
                # k-major over the group's 4 batch tiles; at the last k-slice
                # finish each bank with its bias matmul and hand it to DVE
                # immediately so pooling overlaps the next banks' matmuls.
                for k in range(KT - 1):
                    for bb in range(GRP):
                        lhsT = xt_sb[(k, g)][:, bb * P : (bb + 1) * P]
                        for o in range(OT):
                            nc.tensor.matmul(
                                psums[bb][o][:],
                                lhsT,
                                wt_sb[k][:, o * NFREE : (o + 1) * NFREE],
                                start=(k == 0),
                                stop=False,
                            )
                k = KT - 1
                for bb in range(GRP):
                    lhsT = xt_sb[(k, g)][:, bb * P : (bb + 1) * P]
                    for o in range(OT):
                        nc.tensor.matmul(
                            psums[bb][o][:],
                            lhsT,
                            wt_sb[k][:, o * NFREE : (o + 1) * NFREE],
                            start=False,
                            stop=False,
                        )
                    for o in range(OT):
                        nc.tensor.matmul(
                            psums[bb][o][:],
                            ones[:],
                            bias_sb[:, o * NFREE : (o + 1) * NFREE],
                            start=False,
                            stop=True,
                        )
                    pooled = pooled_pool.tile(
                        [P, OT, P], f32, tag="pooled", name=f"pooled_{g}_{bb}"
                    )
                    for o in range(OT):
                        nc.vector.reduce_max(
                            pooled[:, o, :],
                            psums[bb][o][:].rearrange("p (q f) -> p q f", f=KS),
                            axis=mybir.AxisListType.X,
                        )
                    col = g * GRP + bb
                    nc.vector.reduce_sum(
                        outsb[:, col : col + 1],
                        pooled[:, :, :],
                        axis=mybir.AxisListType.XY,
                    )

            nc.sync.dma_start(out_d[:, :], outsb[:])

    nc.compile()
    _dedup_ldweights(nc)
    return nc


def make_in_maps(x, W, b):
    """Host-side shard + preprocess: transpose, fold 0.5, cast bf16."""
    x = np.asarray(x, dtype=np.float32)
    W = np.asarray(W, dtype=np.float32)
    b = np.asarray(b, dtype=np.float32)

    xt = np.ascontiguousarray(x.T).astype(ml_dtypes.bfloat16)  # [I, B]
    wt = np.ascontiguousarray(W.T * np.float32(SCALE)).astype(
        ml_dtypes.bfloat16
    )  # [I, O]
    bias = (b * np.float32(SCALE)).astype(ml_dtypes.bfloat16).reshape(1, O)

    # Per-batch-half x slabs: [KT, NG, P, GRP*P]
    x_slabs = []
    for g in range(NB_G):
        xg = xt[:, g * BC : (g + 1) * BC]  # [I, BC]
        xr = np.ascontiguousarray(
            xg.reshape(KT, P, NG, GRP * P).transpose(0, 2, 1, 3)
        )
        x_slabs.append(xr)
    # Per-out-feature-quarter W slabs: [KT, P, OC]
    w_slabs = []
    b_slabs = []
    for j in range(NO_G):
        w_slabs.append(
            np.ascontiguousarray(wt[:, j * OC : (j + 1) * OC]).reshape(KT, P, OC)
        )
        b_slabs.append(np.ascontiguousarray(bias[:, j * OC : (j + 1) * OC]))

    in_maps = []
    for c in range(NB_G * NO_G):
        g, j = divmod(c, NO_G)
        in_maps.append({"xt": x_slabs[g], "wt": w_slabs[j], "bias": b_slabs[j]})
    return in_maps


def combine_outputs(results):
    """Sum the 4 out-feature partials per batch half -> full [B] output."""
    out = np.zeros(B, dtype=np.float32)
    for c, r in enumerate(results):
        g = c // NO_G
        part = np.asarray(r["out"], dtype=np.float32)  # [P, BT]
        out[g * BC : (g + 1) * BC] += part.T.reshape(BC)
    return out


def kernel(x, W, b):
    from concourse.bass_utils import run_bass_kernel_spmd

    if "nc" not in _NC_CACHE:
        _NC_CACHE["nc"] = build_bass()
    nc = _NC_CACHE["nc"]
    in_maps = make_in_maps(x, W, b)
    res = run_bass_kernel_spmd(nc, in_maps, core_ids=list(range(NB_G * NO_G)))
    return combine_outputs(res.results)
